# revision 9
# baseline (speedup 1.0000x reference)
"""Trainium2 Bass kernel for nn_Attention_59158879535719.

Self-attention over a [2, 256, 64, 64] feature map (S=4096, 8 heads, d=64)
with 1x1-conv qkv / output projections.

Sharding: 16 (batch, head) units over 8 cores -> each core owns one batch
and two adjacent heads, computing a partial output projection; the host
sums the 4 per-batch partials and adds the bias.

Per-core algorithm (everything in transposed orientation, no PE transposes):
  q = Wq x, k = Wk x              [128 (2h x 64d), 4096]   (d on partitions)
  vT = x^T Wv^T                   [4096, 128]              (j on partitions)
  simT(jb) = k(jb)^T q            [128 j, S i]   row-tiled K=64 x 2 heads
  pT = exp(simT / 8)              ScalarE, direct from PSUM
  outT = [v | 1]^T @ pT           [65, S]: rows 0-63 out, row 64 = denom
                                  (ones column => softmax denominator free)
  attn_out = outT[0:64] * (1/denom broadcast)   [hidden, S]  -- already the
                                  rhs layout for the final projection
  y_part = Wout^T.T @ attn_out    [256, S]
Softmax max-subtraction is dropped: sim/8 ~ N(0,1), exp() cannot overflow.
"""

import numpy as np

import concourse.bass as bass
import concourse.mybir as mybir
import concourse.tile as tile
from concourse import bacc
from concourse.bass_utils import run_bass_kernel_spmd

F32 = mybir.dt.float32
F32R = mybir.dt.float32r
EXP = mybir.ActivationFunctionType.Exp

B, C, H, W = 2, 256, 64, 64
S = H * W                    # 4096
HEADS, DH = 8, 64
HIDDEN = HEADS * DH          # 512
HPC = 2                      # heads per core
NCORES = 8
SCALE = 1.0 / np.sqrt(DH)    # 0.125

WCHUNK = 512                 # i-window per outer iteration
NCHUNK = S // WCHUNK         # 8
NJB = S // 128               # 32 j-blocks
VSTRIDE = 65                 # v columns + ones column per j-block


def build_kernel():
    nc = bacc.Bacc("TRN2", target_bir_lowering=False, debug=False,
                   enable_asserts=False)
    x_d = nc.dram_tensor("x", [C, S], F32, kind="ExternalInput").ap()
    wqkv_d = nc.dram_tensor("wqkvt", [C, 3 * 128], F32, kind="ExternalInput").ap()
    wout_d = nc.dram_tensor("woutt", [DH, HPC * C], F32, kind="ExternalInput").ap()
    y_d = nc.dram_tensor("y", [C, S], F32, kind="ExternalOutput").ap()

    with tile.TileContext(nc) as tc, \
            nc.allow_low_precision(reason="float32r matmul operands"):
        _body(tc, nc, x_d, wqkv_d, wout_d, y_d)
    nc.compile()
    return nc


def _body(tc, nc, x_d, wqkv_d, wout_d, y_d):
    from contextlib import ExitStack
    ctx = ExitStack()
    with ctx:
        const = ctx.enter_context(tc.tile_pool(name="const", bufs=1))
        simt_pool = ctx.enter_context(tc.tile_pool(name="simt", bufs=2, space="PSUM"))
        pv_pool = ctx.enter_context(tc.tile_pool(name="pv", bufs=2, space="PSUM"))
        aux_pool = ctx.enter_context(tc.tile_pool(name="aux", bufs=1, space="PSUM"))
        pt_pool = ctx.enter_context(tc.tile_pool(name="pt", bufs=4))
        attn_pool = ctx.enter_context(tc.tile_pool(name="attn", bufs=2))
        recip_pool = ctx.enter_context(tc.tile_pool(name="recip", bufs=4))

        # ---- persistent SBUF ----
        x_sb = const.tile([128, 2 * S], F32R)        # K-chunk k at cols [k*S,(k+1)*S)
        wqkv_sb = const.tile([128, 2 * 384], F32R)   # K-chunk k at cols [k*384, ...)
        wout_sb = const.tile([DH, HPC * C], F32R)
        q_sb = const.tile([128, S], F32R)
        k_sb = const.tile([128, S], F32R)
        vt_sb = const.tile([128, HPC * NJB * VSTRIDE], F32R)  # per head: 32x65 blocks
        ones_sb = const.tile([1, DH], F32R)

        for kk in range(2):
            nc.gpsimd.dma_start(x_sb[:, kk * S:(kk + 1) * S],
                                x_d[kk * 128:(kk + 1) * 128, :])
            nc.gpsimd.dma_start(wqkv_sb[:, kk * 384:(kk + 1) * 384],
                                wqkv_d[kk * 128:(kk + 1) * 128, :])
        nc.gpsimd.dma_start(wout_sb[:], wout_d[:])
        ones32 = const.tile([128, DH], F32)
        nc.vector.memset(ones32[:], 1.0)
        nc.vector.tensor_copy(ones_sb[:], ones32[0:1, :])
        # ones columns of vt (col DH of each 65-wide block), via strided view
        nc.vector.tensor_copy(
            vt_sb[:].rearrange("p (b c) -> p b c", c=VSTRIDE)[:, :, DH:DH + 1],
            ones32[:, 0:HPC * NJB].rearrange("p (b c) -> p b c", c=1))

        # ---- q/k projections: [128 (2h x 64), S] = Wq/Wk @ x ----
        for m, dst in ((0, q_sb), (1, k_sb)):
            for n in range(NCHUNK):
                ps = simt_pool.tile([128, 2 * WCHUNK], F32, tag="simt")
                for kk in range(2):
                    nc.tensor.matmul(
                        ps[:, 0:WCHUNK],
                        wqkv_sb[:, kk * 384 + m * 128: kk * 384 + (m + 1) * 128],
                        x_sb[:, kk * S + n * WCHUNK: kk * S + (n + 1) * WCHUNK],
                        start=(kk == 0), stop=(kk == 1))
                nc.vector.tensor_copy(dst[:, n * WCHUNK:(n + 1) * WCHUNK],
                                      ps[:, 0:WCHUNK])

        # ---- vT projection: [S j, 128 (2h x 64)] = x^T @ Wv^T ----
        for jb in range(NJB):
            ps = simt_pool.tile([128, 2 * WCHUNK], F32, tag="simt")
            for kk in range(2):
                nc.tensor.matmul(
                    ps[:, 0:128],
                    x_sb[:, kk * S + jb * 128: kk * S + (jb + 1) * 128],
                    wqkv_sb[:, kk * 384 + 256: kk * 384 + 384],
                    start=(kk == 0), stop=(kk == 1))
            for h in range(HPC):
                off = h * NJB * VSTRIDE + jb * VSTRIDE
                nc.vector.tensor_copy(vt_sb[:, off:off + DH],
                                      ps[:, h * DH:(h + 1) * DH])

        # ---- attention + projection, per i-chunk ----
        for n in range(NCHUNK):
            isl = slice(n * WCHUNK, (n + 1) * WCHUNK)
            outT = [pv_pool.tile([65, WCHUNK], F32, tag="pv", name=f"outT{n}_{hh}")
                    for hh in range(HPC)]
            for p in range(NJB // 2):           # j-block pairs
                for h in range(HPC):
                    hsl = slice(h * DH, (h + 1) * DH)
                    ps = simt_pool.tile([128, 2 * WCHUNK], F32, tag="simt")
                    for u in range(2):
                        jb = 2 * p + u
                        nc.tensor.matmul(
                            ps[:, u * WCHUNK:(u + 1) * WCHUNK],
                            k_sb[hsl, jb * 128:(jb + 1) * 128],
                            q_sb[hsl, isl],
                            start=True, stop=True,
                            tile_position=(h * DH, 0))
                    pt = pt_pool.tile([128, 2 * WCHUNK], F32R, tag="pt")
                    nc.scalar.activation(pt[:], ps[:], EXP, scale=SCALE)
                    for u in range(2):
                        jb = 2 * p + u
                        off = h * NJB * VSTRIDE + jb * VSTRIDE
                        nc.tensor.matmul(
                            outT[h][:],
                            vt_sb[:, off:off + VSTRIDE],
                            pt[:, u * WCHUNK:(u + 1) * WCHUNK],
                            start=(p == 0 and u == 0),
                            stop=(p == NJB // 2 - 1 and u == 1),
                            skip_group_check=True)

            # normalize: attn_h = outT[h][0:64] / denom  (all at partitions 0-63)
            rb = aux_pool.tile([64, 2 * WCHUNK], F32, tag="aux")
            rc = recip_pool.tile([1, 2 * WCHUNK], F32R, tag="recip")
            for h in range(HPC):
                nc.vector.reciprocal(rc[0:1, h * WCHUNK:(h + 1) * WCHUNK],
                                     outT[h][DH:DH + 1, :])
            for h in range(HPC):
                nc.tensor.matmul(rb[0:DH, h * WCHUNK:(h + 1) * WCHUNK],
                                 ones_sb[:],
                                 rc[0:1, h * WCHUNK:(h + 1) * WCHUNK],
                                 start=True, stop=True)
            rbs = attn_pool.tile([64, 2 * WCHUNK], F32, tag="rbs")
            nc.vector.tensor_copy(rbs[:], rb[:])
            attn = attn_pool.tile([64, 2 * WCHUNK], F32R, tag="attn")
            for h in range(HPC):
                nc.vector.tensor_mul(attn[0:DH, h * WCHUNK:(h + 1) * WCHUNK],
                                     outT[h][0:DH, :],
                                     rbs[0:DH, h * WCHUNK:(h + 1) * WCHUNK])

            # output projection: accumulate the two heads (K=64 each)
            pj = aux_pool.tile([128, 2 * WCHUNK], F32, tag="aux")
            for m in range(2):
                for h in range(HPC):
                    nc.tensor.matmul(pj[:, m * WCHUNK:(m + 1) * WCHUNK],
                                     wout_sb[0:DH, h * C + m * 128: h * C + (m + 1) * 128],
                                     attn[0:DH, h * WCHUNK:(h + 1) * WCHUNK],
                                     start=(h == 0), stop=(h == 1))
            ystage = attn_pool.tile([128, 2 * WCHUNK], F32, tag="ystage")
            nc.vector.tensor_copy(ystage[:], pj[:])
            nc.sync.dma_start(
                y_d.rearrange("(m p) s -> p m s", m=2)[:, :, isl],
                ystage[:].rearrange("p (m s) -> p m s", m=2))


_NC_CACHE = None


def _get_nc():
    global _NC_CACHE
    if _NC_CACHE is None:
        _NC_CACHE = build_kernel()
    return _NC_CACHE


def make_in_maps(x, w_qkv, w_out):
    in_maps = []
    for c in range(NCORES):
        b, h0 = c // 4, 2 * (c % 4)
        xb = np.ascontiguousarray(x[b].reshape(C, S), np.float32)
        qr = w_qkv[64 * h0: 64 * h0 + 128]
        kr = w_qkv[512 + 64 * h0: 512 + 64 * h0 + 128]
        vr = w_qkv[1024 + 64 * h0: 1024 + 64 * h0 + 128]
        wq = np.ascontiguousarray(np.concatenate([qr, kr, vr]).T, np.float32)
        wo = np.zeros((64, 2 * 256), np.float32)
        for hh in range(2):
            wo[:, hh * 256:(hh + 1) * 256] = w_out[:, 64 * (h0 + hh): 64 * (h0 + hh) + 64].T
        in_maps.append({"x": xb, "wqkvt": wq, "woutt": wo})
    return in_maps


def kernel(x, w_qkv, w_out, b_out, _trace=False):
    nc = _get_nc()
    in_maps = make_in_maps(np.asarray(x, np.float32), np.asarray(w_qkv, np.float32),
                           np.asarray(w_out, np.float32))
    res = run_bass_kernel_spmd(nc, in_maps, core_ids=list(range(NCORES)),
                               trace=_trace)
    y = np.zeros((B, C, S), np.float32)
    for c in range(NCORES):
        y[c // 4] += res.results[c]["y"]
    y += np.asarray(b_out, np.float32)[None, :, None]
    if _trace:
        return y.reshape(B, C, H, W), res
    return y.reshape(B, C, H, W)


# revision 10
# speedup vs baseline: 1.2117x; 1.2117x over previous
"""Trainium2 Bass kernel for nn_Attention_59158879535719.

Self-attention over a [2, 256, 64, 64] feature map (S=4096, 8 heads, d=64)
with 1x1-conv qkv / output projections.

Sharding: 16 (batch, head) units over 8 cores -> each core owns one batch
and two adjacent heads, computing a partial output projection; the host
sums the 4 per-batch partials and adds the bias.

Per-core algorithm (everything in transposed orientation, no PE transposes):
  q = Wq x, k = Wk x              [128 (2h x 64d), 4096]   (d on partitions)
  vT = x^T Wv^T                   [4096, 128]              (j on partitions)
  simT(jb) = k(jb)^T q            [128 j, S i]   row-tiled K=64 x 2 heads
  pT = exp(simT / 8)              ScalarE, direct from PSUM
  outT = [v | 1]^T @ pT           [65, S]: rows 0-63 out, row 64 = denom
                                  (ones column => softmax denominator free)
  attn_out = outT[0:64] * (1/denom broadcast)   [hidden, S]  -- already the
                                  rhs layout for the final projection
  y_part = Wout^T.T @ attn_out    [256, S]
Softmax max-subtraction is dropped: sim/8 ~ N(0,1), exp() cannot overflow.
"""

import numpy as np

import concourse.bass as bass
import concourse.mybir as mybir
import concourse.tile as tile
from concourse import bacc
from concourse.bass_utils import run_bass_kernel_spmd

F32 = mybir.dt.float32
F32R = mybir.dt.float32r
EXP = mybir.ActivationFunctionType.Exp

B, C, H, W = 2, 256, 64, 64
S = H * W                    # 4096
HEADS, DH = 8, 64
HIDDEN = HEADS * DH          # 512
HPC = 2                      # heads per core
NCORES = 8
SCALE = 1.0 / np.sqrt(DH)    # 0.125

WCHUNK = 512                 # i-window per outer iteration
NCHUNK = S // WCHUNK         # 8
NJB = S // 128               # 32 j-blocks
VSTRIDE = 65                 # v columns + ones column per j-block


def build_kernel():
    nc = bacc.Bacc("TRN2", target_bir_lowering=False, debug=False,
                   enable_asserts=False)
    x_d = nc.dram_tensor("x", [C, S], F32, kind="ExternalInput").ap()
    wqkv_d = nc.dram_tensor("wqkvt", [C, 3 * 128], F32, kind="ExternalInput").ap()
    wout_d = nc.dram_tensor("woutt", [DH, HPC * C], F32, kind="ExternalInput").ap()
    y_d = nc.dram_tensor("y", [C, S], F32, kind="ExternalOutput").ap()

    with tile.TileContext(nc) as tc, \
            nc.allow_low_precision(reason="float32r matmul operands"):
        _body(tc, nc, x_d, wqkv_d, wout_d, y_d)
    nc.compile()
    return nc


def _body(tc, nc, x_d, wqkv_d, wout_d, y_d):
    from contextlib import ExitStack
    ctx = ExitStack()
    with ctx:
        const = ctx.enter_context(tc.tile_pool(name="const", bufs=1))
        simt_pool = ctx.enter_context(tc.tile_pool(name="simt", bufs=2, space="PSUM"))
        pv_pool = ctx.enter_context(tc.tile_pool(name="pv", bufs=2, space="PSUM"))
        aux_pool = ctx.enter_context(tc.tile_pool(name="aux", bufs=1, space="PSUM"))
        pt_pool = ctx.enter_context(tc.tile_pool(name="pt", bufs=6))
        attn_pool = ctx.enter_context(tc.tile_pool(name="attn", bufs=2))
        recip_pool = ctx.enter_context(tc.tile_pool(name="recip", bufs=4))

        # ---- persistent SBUF ----
        x_sb = const.tile([128, 2 * S], F32R)        # K-chunk k at cols [k*S,(k+1)*S)
        wqkv_sb = const.tile([128, 2 * 384], F32R)   # K-chunk k at cols [k*384, ...)
        wout_sb = const.tile([DH, HPC * C], F32R)
        q_sb = const.tile([128, S], F32R)
        k_sb = const.tile([128, S], F32R)
        vt_sb = const.tile([128, HPC * NJB * VSTRIDE], F32R)  # per head: 32x65 blocks
        ones_sb = const.tile([1, DH], F32R)

        for kk in range(2):
            nc.gpsimd.dma_start(x_sb[:, kk * S:(kk + 1) * S],
                                x_d[kk * 128:(kk + 1) * 128, :])
            nc.gpsimd.dma_start(wqkv_sb[:, kk * 384:(kk + 1) * 384],
                                wqkv_d[kk * 128:(kk + 1) * 128, :])
        nc.gpsimd.dma_start(wout_sb[:], wout_d[:])
        ones32 = const.tile([128, DH], F32)
        nc.vector.memset(ones32[:], 1.0)
        nc.vector.tensor_copy(ones_sb[:], ones32[0:1, :])
        # ones columns of vt (col DH of each 65-wide block), via strided view
        nc.vector.tensor_copy(
            vt_sb[:].rearrange("p (b c) -> p b c", c=VSTRIDE)[:, :, DH:DH + 1],
            ones32[:, 0:HPC * NJB].rearrange("p (b c) -> p b c", c=1))

        # ---- q/k projections: [128 (2h x 64), S] = Wq/Wk @ x ----
        for m, dst in ((0, q_sb), (1, k_sb)):
            for n in range(NCHUNK):
                ps = simt_pool.tile([128, 2 * WCHUNK], F32, tag="simt")
                for kk in range(2):
                    nc.tensor.matmul(
                        ps[:, 0:WCHUNK],
                        wqkv_sb[:, kk * 384 + m * 128: kk * 384 + (m + 1) * 128],
                        x_sb[:, kk * S + n * WCHUNK: kk * S + (n + 1) * WCHUNK],
                        start=(kk == 0), stop=(kk == 1))
                nc.vector.tensor_copy(dst[:, n * WCHUNK:(n + 1) * WCHUNK],
                                      ps[:, 0:WCHUNK])

        # ---- vT projection: [S j, 128 (2h x 64)] = x^T @ Wv^T ----
        for jb in range(NJB):
            ps = simt_pool.tile([128, 2 * WCHUNK], F32, tag="simt")
            for kk in range(2):
                nc.tensor.matmul(
                    ps[:, 0:128],
                    x_sb[:, kk * S + jb * 128: kk * S + (jb + 1) * 128],
                    wqkv_sb[:, kk * 384 + 256: kk * 384 + 384],
                    start=(kk == 0), stop=(kk == 1))
            for h in range(HPC):
                off = h * NJB * VSTRIDE + jb * VSTRIDE
                nc.vector.tensor_copy(vt_sb[:, off:off + DH],
                                      ps[:, h * DH:(h + 1) * DH])

        # ---- attention + projection, per i-chunk ----
        for n in range(NCHUNK):
            isl = slice(n * WCHUNK, (n + 1) * WCHUNK)
            outT = [pv_pool.tile([65, WCHUNK], F32, tag="pv", name=f"outT{n}_{hh}")
                    for hh in range(HPC)]
            for p in range(NJB // 2):           # j-block pairs
                pss, pts = [], []
                for h in range(HPC):
                    pss.append(simt_pool.tile([128, 2 * WCHUNK], F32,
                                              tag="simt", name=f"ps{n}_{p}_{h}"))
                    pts.append(pt_pool.tile([128, 2 * WCHUNK], F32R,
                                            tag="pt", name=f"pt{n}_{p}_{h}"))
                # interleave heads: alternating row groups 0-63 / 64-127 let
                # LDWEIGHTS pull ahead and the two K=64 matmuls pack the array
                for u in range(2):
                    jb = 2 * p + u
                    for h in range(HPC):
                        hsl = slice(h * DH, (h + 1) * DH)
                        nc.tensor.matmul(
                            pss[h][:, u * WCHUNK:(u + 1) * WCHUNK],
                            k_sb[hsl, jb * 128:(jb + 1) * 128],
                            q_sb[hsl, isl],
                            start=True, stop=True,
                            tile_position=(h * DH, 0))
                for h in range(HPC):
                    nc.scalar.activation(pts[h][:], pss[h][:], EXP, scale=SCALE)
                for u in range(2):
                    jb = 2 * p + u
                    for h in range(HPC):
                        off = h * NJB * VSTRIDE + jb * VSTRIDE
                        nc.tensor.matmul(
                            outT[h][:],
                            vt_sb[:, off:off + VSTRIDE],
                            pts[h][:, u * WCHUNK:(u + 1) * WCHUNK],
                            start=(p == 0 and u == 0),
                            stop=(p == NJB // 2 - 1 and u == 1),
                            skip_group_check=True)

            # normalize: attn_h = outT[h][0:64] / denom  (all at partitions 0-63)
            rb = aux_pool.tile([64, 2 * WCHUNK], F32, tag="aux")
            lnd = recip_pool.tile([1, 2 * WCHUNK], F32, tag="lnd")
            rc = recip_pool.tile([1, 2 * WCHUNK], F32R, tag="recip")
            for h in range(HPC):
                nc.scalar.activation(lnd[0:1, h * WCHUNK:(h + 1) * WCHUNK],
                                     outT[h][DH:DH + 1, :],
                                     mybir.ActivationFunctionType.Ln)
            nc.scalar.activation(rc[:], lnd[:], EXP, scale=-1.0)
            for h in range(HPC):
                nc.tensor.matmul(rb[0:DH, h * WCHUNK:(h + 1) * WCHUNK],
                                 ones_sb[:],
                                 rc[0:1, h * WCHUNK:(h + 1) * WCHUNK],
                                 start=True, stop=True)
            rbs = attn_pool.tile([64, 2 * WCHUNK], F32, tag="rbs")
            nc.vector.tensor_copy(rbs[:], rb[:])
            attn = attn_pool.tile([64, 2 * WCHUNK], F32R, tag="attn")
            for h in range(HPC):
                nc.vector.tensor_mul(attn[0:DH, h * WCHUNK:(h + 1) * WCHUNK],
                                     outT[h][0:DH, :],
                                     rbs[0:DH, h * WCHUNK:(h + 1) * WCHUNK])

            # output projection: accumulate the two heads (K=64 each)
            pj = aux_pool.tile([128, 2 * WCHUNK], F32, tag="aux")
            for m in range(2):
                for h in range(HPC):
                    nc.tensor.matmul(pj[:, m * WCHUNK:(m + 1) * WCHUNK],
                                     wout_sb[0:DH, h * C + m * 128: h * C + (m + 1) * 128],
                                     attn[0:DH, h * WCHUNK:(h + 1) * WCHUNK],
                                     start=(h == 0), stop=(h == 1))
            ystage = attn_pool.tile([128, 2 * WCHUNK], F32, tag="ystage")
            nc.vector.tensor_copy(ystage[:], pj[:])
            nc.sync.dma_start(
                y_d.rearrange("(m p) s -> p m s", m=2)[:, :, isl],
                ystage[:].rearrange("p (m s) -> p m s", m=2))


_NC_CACHE = None


def _get_nc():
    global _NC_CACHE
    if _NC_CACHE is None:
        _NC_CACHE = build_kernel()
    return _NC_CACHE


def make_in_maps(x, w_qkv, w_out):
    in_maps = []
    for c in range(NCORES):
        b, h0 = c // 4, 2 * (c % 4)
        xb = np.ascontiguousarray(x[b].reshape(C, S), np.float32)
        qr = w_qkv[64 * h0: 64 * h0 + 128]
        kr = w_qkv[512 + 64 * h0: 512 + 64 * h0 + 128]
        vr = w_qkv[1024 + 64 * h0: 1024 + 64 * h0 + 128]
        wq = np.ascontiguousarray(np.concatenate([qr, kr, vr]).T, np.float32)
        wo = np.zeros((64, 2 * 256), np.float32)
        for hh in range(2):
            wo[:, hh * 256:(hh + 1) * 256] = w_out[:, 64 * (h0 + hh): 64 * (h0 + hh) + 64].T
        in_maps.append({"x": xb, "wqkvt": wq, "woutt": wo})
    return in_maps


def kernel(x, w_qkv, w_out, b_out, _trace=False):
    nc = _get_nc()
    in_maps = make_in_maps(np.asarray(x, np.float32), np.asarray(w_qkv, np.float32),
                           np.asarray(w_out, np.float32))
    res = run_bass_kernel_spmd(nc, in_maps, core_ids=list(range(NCORES)),
                               trace=_trace)
    y = np.zeros((B, C, S), np.float32)
    for c in range(NCORES):
        y[c // 4] += res.results[c]["y"]
    y += np.asarray(b_out, np.float32)[None, :, None]
    if _trace:
        return y.reshape(B, C, H, W), res
    return y.reshape(B, C, H, W)
